# revision 20
# baseline (speedup 1.0000x reference)
"""Trainium2 Bass kernel for the dense MLP:

    h1  = relu(x @ W1.T + b1)         x:[B,D] W1:[HID,D]
    out = [x, h1] @ W2.T + b2         W2:[OUT, D+HID]

Strategy: data-parallel over the batch across 8 NeuronCores (512 rows
each), weights replicated.  All matmuls run in bf16 with fp32 PSUM
accumulation.  Per core:

  phase 1: h1T tiles [128h x 512b] = W1R_tile.T @ xT_tile, accumulated
           over the 32 k-tiles of D, then bias+ReLU via ScalarE straight
           into a resident SBUF buffer (no DRAM round-trip for h1).
  phase 2: out tiles [128b x 500o] accumulated over the 160 k-tiles of
           D+HID, reading lhsT slices from the resident xT/h1T SBUF
           buffers and streaming W2 tiles.

Host side pre-transposes/reorders x, W1, W2 (and casts to bf16) so every
device DMA is a plain contiguous load, and adds b2 to the gathered
output.
"""

import numpy as np
import ml_dtypes

import concourse.bacc as bacc
import concourse.mybir as mybir
import concourse.tile as tile
from concourse.bass_utils import run_bass_kernel_spmd

B, D, HID, OUT = 4096, 4096, 16384, 1000
NCORES = 8
BC = B // NCORES  # rows of x per core

bf16 = mybir.dt.bfloat16
f32 = mybir.dt.float32
nbf = ml_dtypes.bfloat16

_cache = {}


def build(d=D, hid=HID, out_n=OUT, bc=BC, w1_bufs=3, w2_bufs=4,
          ps1_bufs=4, ps2_bufs=2, kb=4, n_w2_prefetch=3):
    """Build + compile the per-core Bass program. Returns the Bacc."""
    kt1 = d // 128          # k-tiles in layer 1
    nh = hid // 128         # h-tiles
    kt2 = (d + hid) // 128  # k-tiles in layer 2
    nb = bc // 128          # b-tiles per core
    ocs = out_n // 2        # output split in two halves (<=512 each)
    assert ocs <= 512
    n_w2_prefetch = min(n_w2_prefetch, w2_bufs - 2, kt2 // kb)

    nc = bacc.Bacc("TRN2", target_bir_lowering=False, debug=False,
                   num_devices=NCORES)

    XT = nc.dram_tensor("xt", [d, bc], bf16, kind="ExternalInput")
    W1R = nc.dram_tensor("w1r", [nh, 128, d], bf16, kind="ExternalInput")
    W2R = nc.dram_tensor("w2r", [kt2, 128, out_n], bf16, kind="ExternalInput")
    B1R = nc.dram_tensor("b1r", [128, nh], f32, kind="ExternalInput")
    OUTT = nc.dram_tensor("out", [bc, out_n], f32, kind="ExternalOutput")

    add_op = mybir.AluOpType.add
    max_op = mybir.AluOpType.max
    # two independent HWDGE rings (qSyncDynamicHW / qScalarDynamicHW)
    rings = [nc.sync, nc.scalar]

    def w2_dma(ring, w2_t, kt0, oh):
        ring.dma_start(
            w2_t[:],
            W2R.ap()[kt0:kt0 + kb, :, oh * ocs:(oh + 1) * ocs]
            .rearrange("kt p o -> p kt o"))

    with tile.TileContext(nc) as tc:
        with (
            tc.tile_pool(name="persist", bufs=1) as persist,
            tc.tile_pool(name="w2", bufs=w2_bufs) as w2p,
        ):
            xt_sb = persist.tile([128, kt1, bc], bf16, tag="xt")
            h1_sb = persist.tile([128, nh, bc], bf16, tag="h1")
            b1_sb = persist.tile([128, nh], f32, tag="b1")
            nc.sync.dma_start(b1_sb[:], B1R.ap()[:])

            with (
                tc.tile_pool(name="w1", bufs=w1_bufs) as w1p,
                tc.tile_pool(name="ps1", bufs=ps1_bufs, space="PSUM") as ps1,
            ):
                # first W1 tile goes out on the scalar ring immediately,
                # split in 4 so the first matmuls can start on the first
                # quarter (subtile deps); x.T chunks alternate rings
                w1_first = w1p.tile([128, d], bf16, name="w1_t")
                for q in range(4):
                    qs = d // 4
                    nc.scalar.dma_start(w1_first[:, q * qs:(q + 1) * qs],
                                        W1R.ap()[0, :, q * qs:(q + 1) * qs])
                for kt in range(kt1):
                    rings[kt % 2].dma_start(
                        xt_sb[:, kt, :], XT.ap()[kt * 128:(kt + 1) * 128, :])
                # ---- phase 1: h1T = relu(W1 @ x_c.T + b1) ----
                w2_pre = []
                for hi in range(nh):
                    if hi == min(8, nh - 1):
                        # prefetch the first W2 batches once the startup
                        # DMA burst has drained; they sit in their slots
                        # until phase 2 so its matmuls start instantly
                        for i in range(n_w2_prefetch):
                            w2_t = w2p.tile([128, kb, ocs], bf16,
                                            name="w2_t")
                            w2_dma(rings[i % 2], w2_t, i * kb, 0)
                            w2_pre.append(w2_t)
                    if hi == 0:
                        w1_t = w1_first
                    else:
                        w1_t = w1p.tile([128, d], bf16, name="w1_t")
                        rings[1 - hi % 2].dma_start(w1_t[:], W1R.ap()[hi])
                    acc = ps1.tile([128, bc], f32)
                    for kt in range(kt1):
                        nc.tensor.matmul(
                            acc[:],
                            w1_t[:, kt * 128:(kt + 1) * 128],
                            xt_sb[:, kt, :],
                            start=(kt == 0), stop=(kt == kt1 - 1),
                        )
                    # fused relu(acc + b1) on DVE, keeping ScalarE free
                    # to pump the weight-stream DMA ring
                    nc.vector.tensor_scalar(
                        h1_sb[:, hi, :], acc[:],
                        b1_sb[:, hi:hi + 1], 0.0, add_op, max_op)

            # ---- phase 2: out = [x, h1] @ W2.T ----
            with (
                tc.tile_pool(name="ps2", bufs=ps2_bufs,
                             space="PSUM") as ps2,
                tc.tile_pool(name="outp", bufs=2) as outp,
            ):
                for oh in range(2):
                    accs = [ps2.tile([128, ocs], f32, tag=f"acc2_{bt}",
                                     name=f"acc2_{oh}_{bt}")
                            for bt in range(nb)]
                    for bi, kt0 in enumerate(range(0, kt2, kb)):
                        if oh == 0 and bi < n_w2_prefetch:
                            w2_t = w2_pre[bi]
                        else:
                            w2_t = w2p.tile([128, kb, ocs], bf16,
                                            name="w2_t")
                            w2_dma(rings[bi % 2], w2_t, kt0, oh)
                        for j in range(kb):
                            kt = kt0 + j
                            for bt in range(nb):
                                if kt < kt1:
                                    lhsT = xt_sb[:, kt,
                                                 bt * 128:bt * 128 + 128]
                                else:
                                    lhsT = h1_sb[:, kt - kt1,
                                                 bt * 128:bt * 128 + 128]
                                nc.tensor.matmul(
                                    accs[bt][:], lhsT, w2_t[:, j, :],
                                    start=(kt == 0), stop=(kt == kt2 - 1),
                                )
                    for bt in range(nb):
                        out_t = outp.tile([128, ocs], f32)
                        # split the tail evictions across DVE and ACT so
                        # they drain in parallel after the last matmul
                        if bt % 2 == 0:
                            nc.vector.tensor_copy(out_t[:], accs[bt][:])
                        else:
                            nc.scalar.activation(
                                out_t[:], accs[bt][:],
                                mybir.ActivationFunctionType.Copy)
                        rings[bt % 2].dma_start(
                            OUTT.ap()[bt * 128:(bt + 1) * 128,
                                      oh * ocs:(oh + 1) * ocs],
                            out_t[:])

    nc.compile()
    return nc


def prep_inputs(x, W1, b1, W2, b2, bc=BC):
    """Host-side cast to bf16 + re-layout so device DMAs are contiguous."""
    d = x.shape[1]
    hid = W1.shape[0]
    out_n = W2.shape[0]
    nh = hid // 128
    kt2 = (d + hid) // 128

    w1b = np.ascontiguousarray(W1).astype(nbf)
    # W1R[hi, p, kt*128+h] = W1[hi*128+h, kt*128+p]
    w1r = np.ascontiguousarray(
        w1b.reshape(nh, 128, d // 128, 128).transpose(0, 3, 2, 1)
    ).reshape(nh, 128, d)

    w2b = np.ascontiguousarray(W2).astype(nbf)
    # W2R[kt, p, o] = W2[o, kt*128+p]
    w2r = np.ascontiguousarray(
        w2b.reshape(out_n, kt2, 128).transpose(1, 2, 0))

    b1r = np.ascontiguousarray(np.asarray(b1, np.float32).reshape(nh, 128).T)

    xb = np.asarray(x).astype(nbf)
    ncores = x.shape[0] // bc
    in_maps = []
    for c in range(ncores):
        xt_c = np.ascontiguousarray(xb[c * bc:(c + 1) * bc].T)
        in_maps.append({"xt": xt_c, "w1r": w1r, "w2r": w2r, "b1r": b1r})
    return in_maps


def kernel(x, W1, b1, W2, b2):
    x = np.asarray(x)
    W1, b1 = np.asarray(W1), np.asarray(b1)
    W2, b2 = np.asarray(W2), np.asarray(b2)

    if "nc" not in _cache:
        _cache["nc"] = build()
    nc = _cache["nc"]

    in_maps = prep_inputs(x, W1, b1, W2, b2)
    res = run_bass_kernel_spmd(nc, in_maps, core_ids=list(range(NCORES)))
    out = np.concatenate([res.results[c]["out"] for c in range(NCORES)],
                         axis=0)
    return out + np.asarray(b2, np.float32)[None, :]


# revision 21
# speedup vs baseline: 1.0036x; 1.0036x over previous
"""Trainium2 Bass kernel for the dense MLP:

    h1  = relu(x @ W1.T + b1)         x:[B,D] W1:[HID,D]
    out = [x, h1] @ W2.T + b2         W2:[OUT, D+HID]

Strategy: data-parallel over the batch across 8 NeuronCores (512 rows
each), weights replicated.  All matmuls run in bf16 with fp32 PSUM
accumulation.  Per core:

  phase 1: h1T tiles [128h x 512b] = W1R_tile.T @ xT_tile, accumulated
           over the 32 k-tiles of D, then bias+ReLU via ScalarE straight
           into a resident SBUF buffer (no DRAM round-trip for h1).
  phase 2: out tiles [128b x 500o] accumulated over the 160 k-tiles of
           D+HID, reading lhsT slices from the resident xT/h1T SBUF
           buffers and streaming W2 tiles.

Host side pre-transposes/reorders x, W1, W2 (and casts to bf16) so every
device DMA is a plain contiguous load, and adds b2 to the gathered
output.
"""

import numpy as np
import ml_dtypes

import concourse.bacc as bacc
import concourse.mybir as mybir
import concourse.tile as tile
from concourse.bass_utils import run_bass_kernel_spmd

B, D, HID, OUT = 4096, 4096, 16384, 1000
NCORES = 8
BC = B // NCORES  # rows of x per core

bf16 = mybir.dt.bfloat16
f32 = mybir.dt.float32
nbf = ml_dtypes.bfloat16

_cache = {}


def build(d=D, hid=HID, out_n=OUT, bc=BC, w1_bufs=3, w2_bufs=4,
          ps1_bufs=4, ps2_bufs=2, kb=4, n_w2_prefetch=3):
    """Build + compile the per-core Bass program. Returns the Bacc."""
    kt1 = d // 128          # k-tiles in layer 1
    nh = hid // 128         # h-tiles
    kt2 = (d + hid) // 128  # k-tiles in layer 2
    nb = bc // 128          # b-tiles per core
    ocs = out_n // 2        # output split in two halves (<=512 each)
    assert ocs <= 512
    n_w2_prefetch = min(n_w2_prefetch, w2_bufs - 2, kt2 // kb)

    nc = bacc.Bacc("TRN2", target_bir_lowering=False, debug=False,
                   num_devices=NCORES)

    XT = nc.dram_tensor("xt", [d, bc], bf16, kind="ExternalInput")
    W1R = nc.dram_tensor("w1r", [nh, 128, d], bf16, kind="ExternalInput")
    W2R = nc.dram_tensor("w2r", [kt2, 128, out_n], bf16, kind="ExternalInput")
    B1R = nc.dram_tensor("b1r", [128, nh], f32, kind="ExternalInput")
    OUTT = nc.dram_tensor("out", [bc, out_n], f32, kind="ExternalOutput")

    add_op = mybir.AluOpType.add
    max_op = mybir.AluOpType.max
    # two independent HWDGE rings (qSyncDynamicHW / qScalarDynamicHW)
    rings = [nc.sync, nc.scalar]

    def w2_dma(ring, w2_t, kt0, oh):
        ring.dma_start(
            w2_t[:],
            W2R.ap()[kt0:kt0 + kb, :, oh * ocs:(oh + 1) * ocs]
            .rearrange("kt p o -> p kt o"))

    with tile.TileContext(nc) as tc:
        with (
            tc.tile_pool(name="persist", bufs=1) as persist,
            tc.tile_pool(name="w2", bufs=w2_bufs) as w2p,
        ):
            xt_sb = persist.tile([128, kt1, bc], bf16, tag="xt")
            h1_sb = persist.tile([128, nh, bc], bf16, tag="h1")
            b1_sb = persist.tile([128, nh], f32, tag="b1")

            with (
                tc.tile_pool(name="w1", bufs=w1_bufs) as w1p,
                tc.tile_pool(name="ps1", bufs=ps1_bufs, space="PSUM") as ps1,
            ):
                # startup critical path: scalar ring carries only the
                # first W1 tiles; sync ring streams x.T; b1 (needed ~20us
                # in, by the first eviction) queues after x.T
                n_lead = min(3, nh, w1_bufs)
                w1_lead = []
                for hi in range(n_lead):
                    w1_t = w1p.tile([128, d], bf16, name="w1_t")
                    nc.scalar.dma_start(w1_t[:], W1R.ap()[hi])
                    w1_lead.append(w1_t)
                for kt in range(kt1):
                    nc.sync.dma_start(
                        xt_sb[:, kt, :], XT.ap()[kt * 128:(kt + 1) * 128, :])
                nc.sync.dma_start(b1_sb[:], B1R.ap()[:])
                # ---- phase 1: h1T = relu(W1 @ x_c.T + b1) ----
                w2_pre = []
                for hi in range(nh):
                    if hi == min(8, nh - 1):
                        # prefetch the first W2 batches once the startup
                        # DMA burst has drained; they sit in their slots
                        # until phase 2 so its matmuls start instantly
                        for i in range(n_w2_prefetch):
                            w2_t = w2p.tile([128, kb, ocs], bf16,
                                            name="w2_t")
                            w2_dma(rings[i % 2], w2_t, i * kb, 0)
                            w2_pre.append(w2_t)
                    if hi < n_lead:
                        w1_t = w1_lead[hi]
                    else:
                        w1_t = w1p.tile([128, d], bf16, name="w1_t")
                        rings[hi % 2].dma_start(w1_t[:], W1R.ap()[hi])
                    acc = ps1.tile([128, bc], f32)
                    for kt in range(kt1):
                        nc.tensor.matmul(
                            acc[:],
                            w1_t[:, kt * 128:(kt + 1) * 128],
                            xt_sb[:, kt, :],
                            start=(kt == 0), stop=(kt == kt1 - 1),
                        )
                    # fused relu(acc + b1) on DVE, keeping ScalarE free
                    # to pump the weight-stream DMA ring
                    nc.vector.tensor_scalar(
                        h1_sb[:, hi, :], acc[:],
                        b1_sb[:, hi:hi + 1], 0.0, add_op, max_op)

            # ---- phase 2: out = [x, h1] @ W2.T ----
            with (
                tc.tile_pool(name="ps2", bufs=ps2_bufs,
                             space="PSUM") as ps2,
                tc.tile_pool(name="outp", bufs=2) as outp,
            ):
                for oh in range(2):
                    accs = [ps2.tile([128, ocs], f32, tag=f"acc2_{bt}",
                                     name=f"acc2_{oh}_{bt}")
                            for bt in range(nb)]
                    for bi, kt0 in enumerate(range(0, kt2, kb)):
                        if oh == 0 and bi < n_w2_prefetch:
                            w2_t = w2_pre[bi]
                        else:
                            w2_t = w2p.tile([128, kb, ocs], bf16,
                                            name="w2_t")
                            w2_dma(rings[bi % 2], w2_t, kt0, oh)
                        for j in range(kb):
                            kt = kt0 + j
                            for bt in range(nb):
                                if kt < kt1:
                                    lhsT = xt_sb[:, kt,
                                                 bt * 128:bt * 128 + 128]
                                else:
                                    lhsT = h1_sb[:, kt - kt1,
                                                 bt * 128:bt * 128 + 128]
                                nc.tensor.matmul(
                                    accs[bt][:], lhsT, w2_t[:, j, :],
                                    start=(kt == 0), stop=(kt == kt2 - 1),
                                )
                    for bt in range(nb):
                        out_t = outp.tile([128, ocs], f32)
                        # split the tail evictions across DVE and ACT so
                        # they drain in parallel after the last matmul
                        if bt % 2 == 0:
                            nc.vector.tensor_copy(out_t[:], accs[bt][:])
                        else:
                            nc.scalar.activation(
                                out_t[:], accs[bt][:],
                                mybir.ActivationFunctionType.Copy)
                        rings[bt % 2].dma_start(
                            OUTT.ap()[bt * 128:(bt + 1) * 128,
                                      oh * ocs:(oh + 1) * ocs],
                            out_t[:])

    nc.compile()
    return nc


def prep_inputs(x, W1, b1, W2, b2, bc=BC):
    """Host-side cast to bf16 + re-layout so device DMAs are contiguous."""
    d = x.shape[1]
    hid = W1.shape[0]
    out_n = W2.shape[0]
    nh = hid // 128
    kt2 = (d + hid) // 128

    w1b = np.ascontiguousarray(W1).astype(nbf)
    # W1R[hi, p, kt*128+h] = W1[hi*128+h, kt*128+p]
    w1r = np.ascontiguousarray(
        w1b.reshape(nh, 128, d // 128, 128).transpose(0, 3, 2, 1)
    ).reshape(nh, 128, d)

    w2b = np.ascontiguousarray(W2).astype(nbf)
    # W2R[kt, p, o] = W2[o, kt*128+p]
    w2r = np.ascontiguousarray(
        w2b.reshape(out_n, kt2, 128).transpose(1, 2, 0))

    b1r = np.ascontiguousarray(np.asarray(b1, np.float32).reshape(nh, 128).T)

    xb = np.asarray(x).astype(nbf)
    ncores = x.shape[0] // bc
    in_maps = []
    for c in range(ncores):
        xt_c = np.ascontiguousarray(xb[c * bc:(c + 1) * bc].T)
        in_maps.append({"xt": xt_c, "w1r": w1r, "w2r": w2r, "b1r": b1r})
    return in_maps


def kernel(x, W1, b1, W2, b2):
    x = np.asarray(x)
    W1, b1 = np.asarray(W1), np.asarray(b1)
    W2, b2 = np.asarray(W2), np.asarray(b2)

    if "nc" not in _cache:
        _cache["nc"] = build()
    nc = _cache["nc"]

    in_maps = prep_inputs(x, W1, b1, W2, b2)
    res = run_bass_kernel_spmd(nc, in_maps, core_ids=list(range(NCORES)))
    out = np.concatenate([res.results[c]["out"] for c in range(NCORES)],
                         axis=0)
    return out + np.asarray(b2, np.float32)[None, :]


# revision 22
# speedup vs baseline: 1.0047x; 1.0011x over previous
"""Trainium2 Bass kernel for the dense MLP:

    h1  = relu(x @ W1.T + b1)         x:[B,D] W1:[HID,D]
    out = [x, h1] @ W2.T + b2         W2:[OUT, D+HID]

Strategy: data-parallel over the batch across 8 NeuronCores (512 rows
each), weights replicated.  All matmuls run in bf16 with fp32 PSUM
accumulation.  Per core:

  phase 1: h1T tiles [128h x 512b] = W1R_tile.T @ xT_tile, accumulated
           over the 32 k-tiles of D, then bias+ReLU via ScalarE straight
           into a resident SBUF buffer (no DRAM round-trip for h1).
  phase 2: out tiles [128b x 500o] accumulated over the 160 k-tiles of
           D+HID, reading lhsT slices from the resident xT/h1T SBUF
           buffers and streaming W2 tiles.

Host side pre-transposes/reorders x, W1, W2 (and casts to bf16) so every
device DMA is a plain contiguous load, and adds b2 to the gathered
output.
"""

import numpy as np
import ml_dtypes

import concourse.bacc as bacc
import concourse.mybir as mybir
import concourse.tile as tile
from concourse.bass_utils import run_bass_kernel_spmd

B, D, HID, OUT = 4096, 4096, 16384, 1000
NCORES = 8
BC = B // NCORES  # rows of x per core

bf16 = mybir.dt.bfloat16
f32 = mybir.dt.float32
nbf = ml_dtypes.bfloat16

_cache = {}


def build(d=D, hid=HID, out_n=OUT, bc=BC, w1_bufs=3, w2_bufs=4,
          ps1_bufs=4, ps2_bufs=2, kb=4, n_w2_prefetch=3):
    """Build + compile the per-core Bass program. Returns the Bacc."""
    kt1 = d // 128          # k-tiles in layer 1
    nh = hid // 128         # h-tiles
    kt2 = (d + hid) // 128  # k-tiles in layer 2
    nb = bc // 128          # b-tiles per core
    ocs = out_n // 2        # output split in two halves (<=512 each)
    assert ocs <= 512
    n_w2_prefetch = min(n_w2_prefetch, w2_bufs - 2, kt2 // kb)

    nc = bacc.Bacc("TRN2", target_bir_lowering=False, debug=False,
                   num_devices=NCORES)

    XT = nc.dram_tensor("xt", [d, bc], bf16, kind="ExternalInput")
    W1R = nc.dram_tensor("w1r", [nh, 128, d], bf16, kind="ExternalInput")
    W2R = nc.dram_tensor("w2r", [kt2, 128, out_n], bf16, kind="ExternalInput")
    B1R = nc.dram_tensor("b1r", [128, nh], f32, kind="ExternalInput")
    OUTT = nc.dram_tensor("out", [bc, out_n], f32, kind="ExternalOutput")

    add_op = mybir.AluOpType.add
    max_op = mybir.AluOpType.max
    # two independent HWDGE rings (qSyncDynamicHW / qScalarDynamicHW)
    rings = [nc.sync, nc.scalar]

    def w2_dma(ring, w2_t, kt0, oh):
        ring.dma_start(
            w2_t[:],
            W2R.ap()[kt0:kt0 + kb, :, oh * ocs:(oh + 1) * ocs]
            .rearrange("kt p o -> p kt o"))

    with tile.TileContext(nc) as tc:
        with (
            tc.tile_pool(name="persist", bufs=1) as persist,
            tc.tile_pool(name="w2", bufs=w2_bufs) as w2p,
            tc.tile_pool(name="pspre", bufs=1, space="PSUM") as pspre,
        ):
            xt_sb = persist.tile([128, kt1, bc], bf16, tag="xt")
            h1_sb = persist.tile([128, nh, bc], bf16, tag="h1")
            b1_sb = persist.tile([128, nh], f32, tag="b1")

            def l2_matmul(accs, kt, w2_col, start, stop):
                for bt in range(nb):
                    if kt < kt1:
                        lhsT = xt_sb[:, kt, bt * 128:bt * 128 + 128]
                    else:
                        lhsT = h1_sb[:, kt - kt1, bt * 128:bt * 128 + 128]
                    nc.tensor.matmul(accs[bt][:], lhsT, w2_col,
                                     start=start, stop=stop)

            # oh=0 accumulators live from kernel start: the layer-2
            # x-part runs FIRST, as compute cover for the x.T/W1 loads
            # (layer 1's first pass over x.T needs 593 GB/s; the x-part
            # only 148 GB/s, so it hides the HBM-bound startup).
            accs0 = [pspre.tile([128, ocs], f32, tag=f"a0_{bt}",
                                name=f"acc2_0_{bt}") for bt in range(nb)]

            # first W2 batch on the scalar ring = the critical path
            w2_t = w2p.tile([128, kb, ocs], bf16, name="w2_t")
            w2_dma(nc.scalar, w2_t, 0, 0)
            # x.T streams on sync; b1 (first needed ~30us in) after it
            for kt in range(kt1):
                nc.sync.dma_start(
                    xt_sb[:, kt, :], XT.ap()[kt * 128:(kt + 1) * 128, :])
            nc.sync.dma_start(b1_sb[:], B1R.ap()[:])

            with (
                tc.tile_pool(name="w1", bufs=w1_bufs) as w1p,
                tc.tile_pool(name="ps1", bufs=ps1_bufs, space="PSUM") as ps1,
            ):
                # W1 lead tiles queue on scalar behind the first W2 batch
                n_lead = min(3, nh, w1_bufs)
                w1_lead = []
                for hi in range(n_lead):
                    w1_t0 = w1p.tile([128, d], bf16, name="w1_t")
                    nc.scalar.dma_start(w1_t0[:], W1R.ap()[hi])
                    w1_lead.append(w1_t0)

                # ---- phase 0: layer-2 x-part, oh=0 (kt 0..kt1) ----
                for bi, kt0 in enumerate(range(0, kt1, kb)):
                    if bi > 0:
                        w2_t = w2p.tile([128, kb, ocs], bf16, name="w2_t")
                        w2_dma(rings[bi % 2], w2_t, kt0, 0)
                    for j in range(kb):
                        kt = kt0 + j
                        l2_matmul(accs0, kt, w2_t[:, j, :],
                                  start=(kt == 0), stop=False)

                # ---- phase 1: h1T = relu(W1 @ x_c.T + b1) ----
                w2_pre = []
                for hi in range(nh):
                    if hi == min(8, nh - 1):
                        # prefetch the first h-part W2 batches so phase 2
                        # resumes instantly at the boundary
                        for i in range(n_w2_prefetch):
                            w2_t = w2p.tile([128, kb, ocs], bf16,
                                            name="w2_t")
                            w2_dma(rings[i % 2], w2_t, kt1 + i * kb, 0)
                            w2_pre.append(w2_t)
                    if hi < n_lead:
                        w1_t = w1_lead[hi]
                    else:
                        w1_t = w1p.tile([128, d], bf16, name="w1_t")
                        rings[hi % 2].dma_start(w1_t[:], W1R.ap()[hi])
                    acc = ps1.tile([128, bc], f32)
                    for kt in range(kt1):
                        nc.tensor.matmul(
                            acc[:],
                            w1_t[:, kt * 128:(kt + 1) * 128],
                            xt_sb[:, kt, :],
                            start=(kt == 0), stop=(kt == kt1 - 1),
                        )
                    # fused relu(acc + b1) on DVE, keeping ScalarE free
                    # to pump the weight-stream DMA ring
                    nc.vector.tensor_scalar(
                        h1_sb[:, hi, :], acc[:],
                        b1_sb[:, hi:hi + 1], 0.0, add_op, max_op)

            # ---- phase 2: h-part of oh=0, then all of oh=1 ----
            with (
                tc.tile_pool(name="ps2", bufs=1, space="PSUM") as ps2,
                tc.tile_pool(name="outp", bufs=2) as outp,
            ):
                def evict(accs, oh):
                    for bt in range(nb):
                        out_t = outp.tile([128, ocs], f32)
                        # split across DVE and ACT so the final
                        # evictions drain in parallel
                        if bt % 2 == 0:
                            nc.vector.tensor_copy(out_t[:], accs[bt][:])
                        else:
                            nc.scalar.activation(
                                out_t[:], accs[bt][:],
                                mybir.ActivationFunctionType.Copy)
                        rings[bt % 2].dma_start(
                            OUTT.ap()[bt * 128:(bt + 1) * 128,
                                      oh * ocs:(oh + 1) * ocs],
                            out_t[:])

                for bi, kt0 in enumerate(range(kt1, kt2, kb)):
                    if bi < n_w2_prefetch:
                        w2_t = w2_pre[bi]
                    else:
                        w2_t = w2p.tile([128, kb, ocs], bf16, name="w2_t")
                        w2_dma(rings[bi % 2], w2_t, kt0, 0)
                    for j in range(kb):
                        kt = kt0 + j
                        l2_matmul(accs0, kt, w2_t[:, j, :],
                                  start=False, stop=(kt == kt2 - 1))
                evict(accs0, 0)

                accs1 = [ps2.tile([128, ocs], f32, tag=f"a1_{bt}",
                                  name=f"acc2_1_{bt}") for bt in range(nb)]
                for bi, kt0 in enumerate(range(0, kt2, kb)):
                    w2_t = w2p.tile([128, kb, ocs], bf16, name="w2_t")
                    w2_dma(rings[bi % 2], w2_t, kt0, 1)
                    for j in range(kb):
                        kt = kt0 + j
                        l2_matmul(accs1, kt, w2_t[:, j, :],
                                  start=(kt == 0), stop=(kt == kt2 - 1))
                evict(accs1, 1)

    nc.compile()
    return nc


def prep_inputs(x, W1, b1, W2, b2, bc=BC):
    """Host-side cast to bf16 + re-layout so device DMAs are contiguous."""
    d = x.shape[1]
    hid = W1.shape[0]
    out_n = W2.shape[0]
    nh = hid // 128
    kt2 = (d + hid) // 128

    w1b = np.ascontiguousarray(W1).astype(nbf)
    # W1R[hi, p, kt*128+h] = W1[hi*128+h, kt*128+p]
    w1r = np.ascontiguousarray(
        w1b.reshape(nh, 128, d // 128, 128).transpose(0, 3, 2, 1)
    ).reshape(nh, 128, d)

    w2b = np.ascontiguousarray(W2).astype(nbf)
    # W2R[kt, p, o] = W2[o, kt*128+p]
    w2r = np.ascontiguousarray(
        w2b.reshape(out_n, kt2, 128).transpose(1, 2, 0))

    b1r = np.ascontiguousarray(np.asarray(b1, np.float32).reshape(nh, 128).T)

    xb = np.asarray(x).astype(nbf)
    ncores = x.shape[0] // bc
    in_maps = []
    for c in range(ncores):
        xt_c = np.ascontiguousarray(xb[c * bc:(c + 1) * bc].T)
        in_maps.append({"xt": xt_c, "w1r": w1r, "w2r": w2r, "b1r": b1r})
    return in_maps


def kernel(x, W1, b1, W2, b2):
    x = np.asarray(x)
    W1, b1 = np.asarray(W1), np.asarray(b1)
    W2, b2 = np.asarray(W2), np.asarray(b2)

    if "nc" not in _cache:
        _cache["nc"] = build()
    nc = _cache["nc"]

    in_maps = prep_inputs(x, W1, b1, W2, b2)
    res = run_bass_kernel_spmd(nc, in_maps, core_ids=list(range(NCORES)))
    out = np.concatenate([res.results[c]["out"] for c in range(NCORES)],
                         axis=0)
    return out + np.asarray(b2, np.float32)[None, :]


# revision 23
# speedup vs baseline: 1.0106x; 1.0059x over previous
"""Trainium2 Bass kernel for the dense MLP:

    h1  = relu(x @ W1.T + b1)         x:[B,D] W1:[HID,D]
    out = [x, h1] @ W2.T + b2         W2:[OUT, D+HID]

Strategy: data-parallel over the batch across 8 NeuronCores (512 rows
each), weights replicated.  All matmuls run in bf16 with fp32 PSUM
accumulation.  Per core:

  phase 1: h1T tiles [128h x 512b] = W1R_tile.T @ xT_tile, accumulated
           over the 32 k-tiles of D, then bias+ReLU via ScalarE straight
           into a resident SBUF buffer (no DRAM round-trip for h1).
  phase 2: out tiles [128b x 500o] accumulated over the 160 k-tiles of
           D+HID, reading lhsT slices from the resident xT/h1T SBUF
           buffers and streaming W2 tiles.

Host side pre-transposes/reorders x, W1, W2 (and casts to bf16) so every
device DMA is a plain contiguous load, and adds b2 to the gathered
output.
"""

import numpy as np
import ml_dtypes

import concourse.bacc as bacc
import concourse.mybir as mybir
import concourse.tile as tile
from concourse.bass_utils import run_bass_kernel_spmd

B, D, HID, OUT = 4096, 4096, 16384, 1000
NCORES = 8
BC = B // NCORES  # rows of x per core

bf16 = mybir.dt.bfloat16
f32 = mybir.dt.float32
nbf = ml_dtypes.bfloat16

_cache = {}


def build(d=D, hid=HID, out_n=OUT, bc=BC, w1_bufs=3, w2_bufs=4,
          ps1_bufs=4, ps2_bufs=2, kb=4, n_w2_prefetch=3):
    """Build + compile the per-core Bass program. Returns the Bacc."""
    kt1 = d // 128          # k-tiles in layer 1
    nh = hid // 128         # h-tiles
    kt2 = (d + hid) // 128  # k-tiles in layer 2
    nb = bc // 128          # b-tiles per core
    ocs = out_n // 2        # output split in two halves (<=512 each)
    assert ocs <= 512
    n_w2_prefetch = min(n_w2_prefetch, w2_bufs - 2, kt2 // kb)

    nc = bacc.Bacc("TRN2", target_bir_lowering=False, debug=False,
                   num_devices=NCORES)

    XT = nc.dram_tensor("xt", [d, bc], bf16, kind="ExternalInput")
    W1R = nc.dram_tensor("w1r", [nh, 128, d], bf16, kind="ExternalInput")
    W2R = nc.dram_tensor("w2r", [kt2, 128, out_n], bf16, kind="ExternalInput")
    B1R = nc.dram_tensor("b1r", [128, nh], f32, kind="ExternalInput")
    OUTT = nc.dram_tensor("out", [bc, out_n], f32, kind="ExternalOutput")

    add_op = mybir.AluOpType.add
    max_op = mybir.AluOpType.max
    # two independent HWDGE rings (qSyncDynamicHW / qScalarDynamicHW)
    rings = [nc.sync, nc.scalar]

    def w2_dma(ring, w2_t, kt0, oh):
        ring.dma_start(
            w2_t[:],
            W2R.ap()[kt0:kt0 + kb, :, oh * ocs:(oh + 1) * ocs]
            .rearrange("kt p o -> p kt o"))

    with tile.TileContext(nc) as tc:
        with (
            tc.tile_pool(name="persist", bufs=1) as persist,
            tc.tile_pool(name="w2", bufs=w2_bufs) as w2p,
            tc.tile_pool(name="pspre", bufs=1, space="PSUM") as pspre,
        ):
            xt_sb = persist.tile([128, kt1, bc], bf16, tag="xt")
            h1_sb = persist.tile([128, nh, bc], bf16, tag="h1")
            b1_sb = persist.tile([128, nh], f32, tag="b1")

            def l2_matmul(accs, kt, w2_col, start, stop):
                for bt in range(nb):
                    if kt < kt1:
                        lhsT = xt_sb[:, kt, bt * 128:bt * 128 + 128]
                    else:
                        lhsT = h1_sb[:, kt - kt1, bt * 128:bt * 128 + 128]
                    nc.tensor.matmul(accs[bt][:], lhsT, w2_col,
                                     start=start, stop=stop)

            # oh=0 accumulators live from kernel start: the layer-2
            # x-part runs FIRST, as compute cover for the x.T/W1 loads
            # (layer 1's first pass over x.T needs 593 GB/s; the x-part
            # only 148 GB/s, so it hides the HBM-bound startup).
            accs0 = [pspre.tile([128, ocs], f32, tag=f"a0_{bt}",
                                name=f"acc2_0_{bt}") for bt in range(nb)]

            # startup queues -- scalar ring: the phase-0 W2 batches (the
            # critical path for the first matmuls); sync ring: first W1
            # tile, then the x.T stream, b1, and the remaining W1 leads
            w2_first = w2p.tile([128, kb, ocs], bf16, name="w2_t")
            w2_dma(nc.scalar, w2_first, 0, 0)

            with (
                tc.tile_pool(name="w1", bufs=w1_bufs) as w1p,
                tc.tile_pool(name="ps1", bufs=ps1_bufs, space="PSUM") as ps1,
            ):
                n_lead = min(3, nh, w1_bufs)
                w1_lead = [w1p.tile([128, d], bf16, name="w1_t")
                           for _ in range(n_lead)]
                nc.sync.dma_start(w1_lead[0][:], W1R.ap()[0])
                for kt in range(kt1):
                    nc.sync.dma_start(
                        xt_sb[:, kt, :], XT.ap()[kt * 128:(kt + 1) * 128, :])
                nc.sync.dma_start(b1_sb[:], B1R.ap()[:])
                for hi in range(1, n_lead):
                    nc.sync.dma_start(w1_lead[hi][:], W1R.ap()[hi])

                # ---- phase 0: layer-2 x-part, oh=0 (kt 0..kt1) ----
                w2_t = w2_first
                for bi, kt0 in enumerate(range(0, kt1, kb)):
                    if bi > 0:
                        w2_t = w2p.tile([128, kb, ocs], bf16, name="w2_t")
                        w2_dma(nc.scalar, w2_t, kt0, 0)
                    for j in range(kb):
                        kt = kt0 + j
                        l2_matmul(accs0, kt, w2_t[:, j, :],
                                  start=(kt == 0), stop=False)

                # ---- phase 1: h1T = relu(W1 @ x_c.T + b1) ----
                w2_pre = []
                for hi in range(nh):
                    if hi == min(8, nh - 1):
                        # prefetch the first h-part W2 batches so phase 2
                        # resumes instantly at the boundary
                        for i in range(n_w2_prefetch):
                            w2_t = w2p.tile([128, kb, ocs], bf16,
                                            name="w2_t")
                            w2_dma(rings[i % 2], w2_t, kt1 + i * kb, 0)
                            w2_pre.append(w2_t)
                    if hi < n_lead:
                        w1_t = w1_lead[hi]
                    else:
                        w1_t = w1p.tile([128, d], bf16, name="w1_t")
                        rings[hi % 2].dma_start(w1_t[:], W1R.ap()[hi])
                    acc = ps1.tile([128, bc], f32)
                    for kt in range(kt1):
                        nc.tensor.matmul(
                            acc[:],
                            w1_t[:, kt * 128:(kt + 1) * 128],
                            xt_sb[:, kt, :],
                            start=(kt == 0), stop=(kt == kt1 - 1),
                        )
                    # fused relu(acc + b1) on DVE, keeping ScalarE free
                    # to pump the weight-stream DMA ring
                    nc.vector.tensor_scalar(
                        h1_sb[:, hi, :], acc[:],
                        b1_sb[:, hi:hi + 1], 0.0, add_op, max_op)

            # ---- phase 2: h-part of oh=0, then all of oh=1 ----
            with (
                tc.tile_pool(name="ps2", bufs=1, space="PSUM") as ps2,
                tc.tile_pool(name="outp", bufs=2) as outp,
            ):
                def evict(accs, oh):
                    for bt in range(nb):
                        out_t = outp.tile([128, ocs], f32)
                        # split across DVE and ACT so the final
                        # evictions drain in parallel
                        if bt % 2 == 0:
                            nc.vector.tensor_copy(out_t[:], accs[bt][:])
                        else:
                            nc.scalar.activation(
                                out_t[:], accs[bt][:],
                                mybir.ActivationFunctionType.Copy)
                        rings[bt % 2].dma_start(
                            OUTT.ap()[bt * 128:(bt + 1) * 128,
                                      oh * ocs:(oh + 1) * ocs],
                            out_t[:])

                for bi, kt0 in enumerate(range(kt1, kt2, kb)):
                    if bi < n_w2_prefetch:
                        w2_t = w2_pre[bi]
                    else:
                        w2_t = w2p.tile([128, kb, ocs], bf16, name="w2_t")
                        w2_dma(rings[bi % 2], w2_t, kt0, 0)
                    for j in range(kb):
                        kt = kt0 + j
                        l2_matmul(accs0, kt, w2_t[:, j, :],
                                  start=False, stop=(kt == kt2 - 1))
                evict(accs0, 0)

                accs1 = [ps2.tile([128, ocs], f32, tag=f"a1_{bt}",
                                  name=f"acc2_1_{bt}") for bt in range(nb)]
                for bi, kt0 in enumerate(range(0, kt2, kb)):
                    w2_t = w2p.tile([128, kb, ocs], bf16, name="w2_t")
                    w2_dma(rings[bi % 2], w2_t, kt0, 1)
                    for j in range(kb):
                        kt = kt0 + j
                        l2_matmul(accs1, kt, w2_t[:, j, :],
                                  start=(kt == 0), stop=(kt == kt2 - 1))
                evict(accs1, 1)

    nc.compile()
    return nc


def prep_inputs(x, W1, b1, W2, b2, bc=BC):
    """Host-side cast to bf16 + re-layout so device DMAs are contiguous."""
    d = x.shape[1]
    hid = W1.shape[0]
    out_n = W2.shape[0]
    nh = hid // 128
    kt2 = (d + hid) // 128

    w1b = np.ascontiguousarray(W1).astype(nbf)
    # W1R[hi, p, kt*128+h] = W1[hi*128+h, kt*128+p]
    w1r = np.ascontiguousarray(
        w1b.reshape(nh, 128, d // 128, 128).transpose(0, 3, 2, 1)
    ).reshape(nh, 128, d)

    w2b = np.ascontiguousarray(W2).astype(nbf)
    # W2R[kt, p, o] = W2[o, kt*128+p]
    w2r = np.ascontiguousarray(
        w2b.reshape(out_n, kt2, 128).transpose(1, 2, 0))

    b1r = np.ascontiguousarray(np.asarray(b1, np.float32).reshape(nh, 128).T)

    xb = np.asarray(x).astype(nbf)
    ncores = x.shape[0] // bc
    in_maps = []
    for c in range(ncores):
        xt_c = np.ascontiguousarray(xb[c * bc:(c + 1) * bc].T)
        in_maps.append({"xt": xt_c, "w1r": w1r, "w2r": w2r, "b1r": b1r})
    return in_maps


def kernel(x, W1, b1, W2, b2):
    x = np.asarray(x)
    W1, b1 = np.asarray(W1), np.asarray(b1)
    W2, b2 = np.asarray(W2), np.asarray(b2)

    if "nc" not in _cache:
        _cache["nc"] = build()
    nc = _cache["nc"]

    in_maps = prep_inputs(x, W1, b1, W2, b2)
    res = run_bass_kernel_spmd(nc, in_maps, core_ids=list(range(NCORES)))
    out = np.concatenate([res.results[c]["out"] for c in range(NCORES)],
                         axis=0)
    return out + np.asarray(b2, np.float32)[None, :]


# revision 24
# speedup vs baseline: 1.0149x; 1.0042x over previous
"""Trainium2 Bass kernel for the dense MLP:

    h1  = relu(x @ W1.T + b1)         x:[B,D] W1:[HID,D]
    out = [x, h1] @ W2.T + b2         W2:[OUT, D+HID]

Strategy: data-parallel over the batch across 8 NeuronCores (512 rows
each), weights replicated.  All matmuls run in bf16 with fp32 PSUM
accumulation.  Per core:

  phase 1: h1T tiles [128h x 512b] = W1R_tile.T @ xT_tile, accumulated
           over the 32 k-tiles of D, then bias+ReLU via ScalarE straight
           into a resident SBUF buffer (no DRAM round-trip for h1).
  phase 2: out tiles [128b x 500o] accumulated over the 160 k-tiles of
           D+HID, reading lhsT slices from the resident xT/h1T SBUF
           buffers and streaming W2 tiles.

Host side pre-transposes/reorders x, W1, W2 (and casts to bf16) so every
device DMA is a plain contiguous load, and adds b2 to the gathered
output.
"""

import numpy as np
import ml_dtypes

import concourse.bacc as bacc
import concourse.mybir as mybir
import concourse.tile as tile
from concourse.bass_utils import run_bass_kernel_spmd

B, D, HID, OUT = 4096, 4096, 16384, 1000
NCORES = 8
BC = B // NCORES  # rows of x per core

bf16 = mybir.dt.bfloat16
f32 = mybir.dt.float32
nbf = ml_dtypes.bfloat16

_cache = {}


def build(d=D, hid=HID, out_n=OUT, bc=BC, w1_bufs=3, w2_bufs=4,
          ps1_bufs=4, ps2_bufs=2, kb=4, n_w2_prefetch=3):
    """Build + compile the per-core Bass program. Returns the Bacc."""
    kt1 = d // 128          # k-tiles in layer 1
    nh = hid // 128         # h-tiles
    kt2 = (d + hid) // 128  # k-tiles in layer 2
    nb = bc // 128          # b-tiles per core
    ocs = out_n // 2        # output split in two halves (<=512 each)
    assert ocs <= 512
    n_w2_prefetch = min(n_w2_prefetch, w2_bufs - 2, kt2 // kb)

    nc = bacc.Bacc("TRN2", target_bir_lowering=False, debug=False,
                   num_devices=NCORES)

    XT = nc.dram_tensor("xt", [d, bc], bf16, kind="ExternalInput")
    W1R = nc.dram_tensor("w1r", [nh, 128, d], bf16, kind="ExternalInput")
    W2R = nc.dram_tensor("w2r", [kt2, 128, out_n], bf16, kind="ExternalInput")
    B1R = nc.dram_tensor("b1r", [128, nh], f32, kind="ExternalInput")
    OUTT = nc.dram_tensor("out", [bc, out_n], f32, kind="ExternalOutput")

    add_op = mybir.AluOpType.add
    max_op = mybir.AluOpType.max
    # two independent HWDGE rings (qSyncDynamicHW / qScalarDynamicHW)
    rings = [nc.sync, nc.scalar]

    def w2_dma(ring, w2_t, kt0, oh):
        ring.dma_start(
            w2_t[:],
            W2R.ap()[kt0:kt0 + kb, :, oh * ocs:(oh + 1) * ocs]
            .rearrange("kt p o -> p kt o"))

    with tile.TileContext(nc) as tc:
        with (
            tc.tile_pool(name="persist", bufs=1) as persist,
            tc.tile_pool(name="w2", bufs=w2_bufs) as w2p,
            tc.tile_pool(name="pspre", bufs=1, space="PSUM") as pspre,
        ):
            xt_sb = persist.tile([128, kt1, bc], bf16, tag="xt")
            h1_sb = persist.tile([128, nh, bc], bf16, tag="h1")
            b1_sb = persist.tile([128, nh], f32, tag="b1")

            def l2_matmul(accs, kt, w2_col, start, stop):
                for bt in range(nb):
                    if kt < kt1:
                        lhsT = xt_sb[:, kt, bt * 128:bt * 128 + 128]
                    else:
                        lhsT = h1_sb[:, kt - kt1, bt * 128:bt * 128 + 128]
                    nc.tensor.matmul(accs[bt][:], lhsT, w2_col,
                                     start=start, stop=stop)

            # oh=0 accumulators live from kernel start: the layer-2
            # x-part runs FIRST, as compute cover for the x.T/W1 loads
            # (layer 1's first pass over x.T needs 593 GB/s; the x-part
            # only 148 GB/s, so it hides the HBM-bound startup).
            accs0 = [pspre.tile([128, ocs], f32, tag=f"a0_{bt}",
                                name=f"acc2_0_{bt}") for bt in range(nb)]

            # startup queues -- scalar ring: the phase-0 W2 batches (the
            # critical path for the first matmuls); sync ring: first W1
            # tile, then the x.T stream, b1, and the remaining W1 leads
            # first W2 batch split so the very first matmuls only wait
            # for its first k-tile (subtile deps)
            w2_first = w2p.tile([128, kb, ocs], bf16, name="w2_t")
            nc.scalar.dma_start(w2_first[:, 0:1, :],
                                W2R.ap()[0:1, :, 0:ocs]
                                .rearrange("kt p o -> p kt o"))
            nc.scalar.dma_start(w2_first[:, 1:kb, :],
                                W2R.ap()[1:kb, :, 0:ocs]
                                .rearrange("kt p o -> p kt o"))

            with (
                tc.tile_pool(name="w1", bufs=w1_bufs) as w1p,
                tc.tile_pool(name="ps1", bufs=ps1_bufs, space="PSUM") as ps1,
            ):
                n_lead = min(3, nh, w1_bufs)
                w1_lead = [w1p.tile([128, d], bf16, name="w1_t")
                           for _ in range(n_lead)]
                # x.T first on sync (its chunk 0 gates the first matmul);
                # W1 leads are not needed until phase 1, ~35us in
                for kt in range(kt1):
                    nc.sync.dma_start(
                        xt_sb[:, kt, :], XT.ap()[kt * 128:(kt + 1) * 128, :])
                nc.sync.dma_start(b1_sb[:], B1R.ap()[:])
                for hi in range(n_lead):
                    nc.sync.dma_start(w1_lead[hi][:], W1R.ap()[hi])

                # ---- phase 0: layer-2 x-part, oh=0 (kt 0..kt1) ----
                w2_t = w2_first
                for bi, kt0 in enumerate(range(0, kt1, kb)):
                    if bi > 0:
                        w2_t = w2p.tile([128, kb, ocs], bf16, name="w2_t")
                        w2_dma(nc.scalar, w2_t, kt0, 0)
                    for j in range(kb):
                        kt = kt0 + j
                        l2_matmul(accs0, kt, w2_t[:, j, :],
                                  start=(kt == 0), stop=False)

                # ---- phase 1: h1T = relu(W1 @ x_c.T + b1) ----
                w2_pre = []
                for hi in range(nh):
                    if hi == min(8, nh - 1):
                        # prefetch the first h-part W2 batches so phase 2
                        # resumes instantly at the boundary
                        for i in range(n_w2_prefetch):
                            w2_t = w2p.tile([128, kb, ocs], bf16,
                                            name="w2_t")
                            w2_dma(rings[i % 2], w2_t, kt1 + i * kb, 0)
                            w2_pre.append(w2_t)
                    if hi < n_lead:
                        w1_t = w1_lead[hi]
                    else:
                        w1_t = w1p.tile([128, d], bf16, name="w1_t")
                        rings[hi % 2].dma_start(w1_t[:], W1R.ap()[hi])
                    acc = ps1.tile([128, bc], f32)
                    for kt in range(kt1):
                        nc.tensor.matmul(
                            acc[:],
                            w1_t[:, kt * 128:(kt + 1) * 128],
                            xt_sb[:, kt, :],
                            start=(kt == 0), stop=(kt == kt1 - 1),
                        )
                    # fused relu(acc + b1) on DVE, keeping ScalarE free
                    # to pump the weight-stream DMA ring
                    nc.vector.tensor_scalar(
                        h1_sb[:, hi, :], acc[:],
                        b1_sb[:, hi:hi + 1], 0.0, add_op, max_op)

            # ---- phase 2: h-part of oh=0, then all of oh=1 ----
            with (
                tc.tile_pool(name="ps2", bufs=1, space="PSUM") as ps2,
                tc.tile_pool(name="outp", bufs=2) as outp,
            ):
                def evict(accs, oh):
                    for bt in range(nb):
                        out_t = outp.tile([128, ocs], f32)
                        # split across DVE and ACT so the final
                        # evictions drain in parallel
                        if bt % 2 == 0:
                            nc.vector.tensor_copy(out_t[:], accs[bt][:])
                        else:
                            nc.scalar.activation(
                                out_t[:], accs[bt][:],
                                mybir.ActivationFunctionType.Copy)
                        rings[bt % 2].dma_start(
                            OUTT.ap()[bt * 128:(bt + 1) * 128,
                                      oh * ocs:(oh + 1) * ocs],
                            out_t[:])

                for bi, kt0 in enumerate(range(kt1, kt2, kb)):
                    if bi < n_w2_prefetch:
                        w2_t = w2_pre[bi]
                    else:
                        w2_t = w2p.tile([128, kb, ocs], bf16, name="w2_t")
                        w2_dma(rings[bi % 2], w2_t, kt0, 0)
                    for j in range(kb):
                        kt = kt0 + j
                        l2_matmul(accs0, kt, w2_t[:, j, :],
                                  start=False, stop=(kt == kt2 - 1))
                evict(accs0, 0)

                accs1 = [ps2.tile([128, ocs], f32, tag=f"a1_{bt}",
                                  name=f"acc2_1_{bt}") for bt in range(nb)]
                for bi, kt0 in enumerate(range(0, kt2, kb)):
                    w2_t = w2p.tile([128, kb, ocs], bf16, name="w2_t")
                    w2_dma(rings[bi % 2], w2_t, kt0, 1)
                    for j in range(kb):
                        kt = kt0 + j
                        l2_matmul(accs1, kt, w2_t[:, j, :],
                                  start=(kt == 0), stop=(kt == kt2 - 1))
                evict(accs1, 1)

    nc.compile()
    return nc


def prep_inputs(x, W1, b1, W2, b2, bc=BC):
    """Host-side cast to bf16 + re-layout so device DMAs are contiguous."""
    d = x.shape[1]
    hid = W1.shape[0]
    out_n = W2.shape[0]
    nh = hid // 128
    kt2 = (d + hid) // 128

    w1b = np.ascontiguousarray(W1).astype(nbf)
    # W1R[hi, p, kt*128+h] = W1[hi*128+h, kt*128+p]
    w1r = np.ascontiguousarray(
        w1b.reshape(nh, 128, d // 128, 128).transpose(0, 3, 2, 1)
    ).reshape(nh, 128, d)

    w2b = np.ascontiguousarray(W2).astype(nbf)
    # W2R[kt, p, o] = W2[o, kt*128+p]
    w2r = np.ascontiguousarray(
        w2b.reshape(out_n, kt2, 128).transpose(1, 2, 0))

    b1r = np.ascontiguousarray(np.asarray(b1, np.float32).reshape(nh, 128).T)

    xb = np.asarray(x).astype(nbf)
    ncores = x.shape[0] // bc
    in_maps = []
    for c in range(ncores):
        xt_c = np.ascontiguousarray(xb[c * bc:(c + 1) * bc].T)
        in_maps.append({"xt": xt_c, "w1r": w1r, "w2r": w2r, "b1r": b1r})
    return in_maps


def kernel(x, W1, b1, W2, b2):
    x = np.asarray(x)
    W1, b1 = np.asarray(W1), np.asarray(b1)
    W2, b2 = np.asarray(W2), np.asarray(b2)

    if "nc" not in _cache:
        _cache["nc"] = build()
    nc = _cache["nc"]

    in_maps = prep_inputs(x, W1, b1, W2, b2)
    res = run_bass_kernel_spmd(nc, in_maps, core_ids=list(range(NCORES)))
    out = np.concatenate([res.results[c]["out"] for c in range(NCORES)],
                         axis=0)
    return out + np.asarray(b2, np.float32)[None, :]
